# revision 2
# baseline (speedup 1.0000x reference)
"""Multi-head attention (QKV proj + softmax attention + out proj) on 8 TRN2 NeuronCores.

Sharding: batch (2) x head-pairs (4) -> 8 cores. Each core computes q,k,v for its
2 heads of its batch, full attention over the 4096-token sequence for those heads,
and a partial output projection (row-sharded W_proj). The host sums the 4 partial
projections per batch and adds b_proj.

Per-core layout choices:
  - scores are computed transposed ([t, s] = keys on partitions), so the exp'd
    probabilities can feed the AV matmul directly as lhsT with no transposes.
  - the softmax denominator comes for free from a ones-column appended to V
    (row 64 of the [65, s] AV accumulator).
  - the two heads' score matmuls run concurrently on PE row-groups 0-1/2-3
    (K=64 each, lhsT/rhs at base partitions 0 and 64).
  - exp runs on ScalarE straight out of 4 PSUM banks in [128, 2048] chunks.
"""
from contextlib import ExitStack

import ml_dtypes
import numpy as np

import concourse.bass as bass
import concourse.tile as tile
from concourse import bacc, mybir
from concourse.bass_utils import run_bass_kernel_spmd

B, S, D = 2, 4096, 512
H, HD = 8, 64
SCALE = HD**-0.5
P = 128
SC = 512            # s-chunk width (query positions per inner block)
N_SC = S // SC      # 8
N_T = S // P        # 32 key chunks
KT = D // P         # 4 contraction tiles for the projections
GT = 2              # key chunks per exp group (4 PSUM banks)
NG = N_T // GT      # 16
VW = 2 * (HD + 1)   # v_ext width: [vA | onesA | vB | onesB]
BF16 = mybir.dt.bfloat16
F32 = mybir.dt.float32
EXP = mybir.ActivationFunctionType.Exp

_NC = None


def _emit(tc, out_d, xT_d, wq_d, wk_d, wv_d, bq_d, bk_d, bv_d, wpa_d, wpb_d):
    nc = tc.nc
    with ExitStack() as ctx:
        consts = ctx.enter_context(tc.tile_pool(name="consts", bufs=1))
        big = ctx.enter_context(tc.tile_pool(name="big", bufs=1))
        xt_pool = ctx.enter_context(tc.tile_pool(name="xt", bufs=3))
        work = ctx.enter_context(tc.tile_pool(name="work", bufs=2))

        wq_sb = consts.tile([P, KT, P], BF16, tag="wq")
        nc.sync.dma_start(wq_sb[:], wq_d.rearrange("(kt p) m -> p kt m", p=P))
        wk_sb = consts.tile([P, KT, P], BF16, tag="wk")
        nc.sync.dma_start(wk_sb[:], wk_d.rearrange("(kt p) m -> p kt m", p=P))
        wv_sb = consts.tile([P, KT, VW], BF16, tag="wv")
        nc.sync.dma_start(wv_sb[:], wv_d.rearrange("(kt p) m -> p kt m", p=P))
        wpa_sb = consts.tile([HD, D], BF16, tag="wpa")
        nc.sync.dma_start(wpa_sb[:], wpa_d)
        wpb_sb = consts.tile([HD, D], BF16, tag="wpb")
        nc.sync.dma_start(wpb_sb[:], wpb_d)
        bq_sb = consts.tile([P, 1], F32, tag="bq")
        nc.sync.dma_start(bq_sb[:], bq_d)
        bk_sb = consts.tile([P, 1], F32, tag="bk")
        nc.sync.dma_start(bk_sb[:], bk_d)
        bv_sb = consts.tile([P, VW], F32, tag="bv")
        nc.sync.dma_start(bv_sb[:], bv_d)
        # row HD (=64) of this is the lhsT of the K=1 broadcast matmul
        ones_sb = consts.tile([P, HD], F32, tag="ones")
        nc.vector.memset(ones_sb[:], 1.0)

        qT_sb = big.tile([P, S], BF16, tag="qT")   # rows 0-63 head A, 64-127 head B
        kT_sb = big.tile([P, S], BF16, tag="kT")
        v_sb = big.tile([P, N_T, VW], BF16, tag="v")
        pAB = big.tile([P, N_T, 2, SC], BF16, tag="p")  # exp'd scoresT per head

        xT_r = xT_d.rearrange("(kt p) s -> p kt s", p=P)

        # ---- Phase 1: q,k (transposed layout) and v_ext projections
        with tc.tile_pool(name="ph1ps", bufs=2, space="PSUM") as ph1:
            for sc in range(N_SC):
                ssl = slice(sc * SC, (sc + 1) * SC)
                xt = xt_pool.tile([P, KT, SC], BF16, tag="xt")
                nc.sync.dma_start(xt[:], xT_r[:, :, ssl])
                qp = ph1.tile([P, SC], F32, tag="qp")
                for kt in range(KT):
                    nc.tensor.matmul(qp[:], lhsT=wq_sb[:, kt, :], rhs=xt[:, kt, :],
                                     start=kt == 0, stop=kt == KT - 1)
                nc.vector.tensor_scalar_add(out=qT_sb[:, ssl], in0=qp[:], scalar1=bq_sb[:])
                kp = ph1.tile([P, SC], F32, tag="kp")
                for kt in range(KT):
                    nc.tensor.matmul(kp[:], lhsT=wk_sb[:, kt, :], rhs=xt[:, kt, :],
                                     start=kt == 0, stop=kt == KT - 1)
                nc.vector.tensor_scalar_add(out=kT_sb[:, ssl], in0=kp[:], scalar1=bk_sb[:])
                for st in range(4):
                    vp = ph1.tile([P, VW], F32, tag="vp")
                    for kt in range(KT):
                        nc.tensor.matmul(vp[:], lhsT=xt[:, kt, st * P:(st + 1) * P],
                                         rhs=wv_sb[:, kt, :],
                                         start=kt == 0, stop=kt == KT - 1)
                    nc.vector.tensor_add(out=v_sb[:, sc * 4 + st, :], in0=vp[:], in1=bv_sb[:])

        # ---- Phase 2: scoresT -> exp -> AV (+denominator) -> normalize -> proj
        sc_ps = ctx.enter_context(tc.tile_pool(name="scps", bufs=1, space="PSUM"))
        av_ps = ctx.enter_context(tc.tile_pool(name="avps", bufs=1, space="PSUM"))
        mi_ps = ctx.enter_context(tc.tile_pool(name="mips", bufs=2, space="PSUM"))

        def scores_group(sc, g):
            ssl = slice(sc * SC, (sc + 1) * SC)
            sAB = sc_ps.tile([P, GT, 2, SC], F32, tag="s")
            for i in range(GT):
                t = g * GT + i
                tsl = slice(t * P, (t + 1) * P)
                nc.tensor.matmul(sAB[:, i, 0, :], lhsT=kT_sb[0:HD, tsl],
                                 rhs=qT_sb[0:HD, ssl], start=True, stop=True)
                nc.tensor.matmul(sAB[:, i, 1, :], lhsT=kT_sb[HD:P, tsl],
                                 rhs=qT_sb[HD:P, ssl], start=True, stop=True)
            nc.scalar.activation(out=pAB[:, g * GT:(g + 1) * GT, :, :], in_=sAB[:],
                                 func=EXP, scale=SCALE)

        def av_group(avA, avB, g):
            for i in range(GT):
                t = g * GT + i
                nc.tensor.matmul(avA[:], lhsT=v_sb[:, t, 0:HD + 1], rhs=pAB[:, t, 0, :],
                                 start=t == 0, stop=t == N_T - 1)
                nc.tensor.matmul(avB[:], lhsT=v_sb[:, t, HD + 1:VW], rhs=pAB[:, t, 1, :],
                                 start=t == 0, stop=t == N_T - 1)

        def norm_head(avX, h):
            # avX: [65, SC] psum; row 64 = sum_t exp. Result: [64, SC] bf16 sbuf.
            rec = work.tile([HD + 1, SC], F32, tag="rec")
            nc.vector.reciprocal(rec[HD:HD + 1, :], avX[HD:HD + 1, :])
            bc_ps = mi_ps.tile([HD, SC], F32, tag="m")
            nc.tensor.matmul(bc_ps[:], lhsT=ones_sb[HD:HD + 1, :], rhs=rec[HD:HD + 1, :],
                             start=True, stop=True)
            bc_sb = work.tile([HD, SC], F32, tag="bc")
            nc.vector.tensor_copy(bc_sb[:], bc_ps[:])
            avn = work.tile([HD, SC], BF16, tag=f"avn{h}")
            nc.vector.tensor_mul(avn[:], avX[0:HD, :], bc_sb[:])
            return avn

        def proj_st(sc, avnA, avnB, st):
            op = mi_ps.tile([P, D], F32, tag="m")
            asl = slice(st * P, (st + 1) * P)
            nc.tensor.matmul(op[:], lhsT=avnA[:, asl], rhs=wpa_sb[:], start=True, stop=False)
            nc.tensor.matmul(op[:], lhsT=avnB[:, asl], rhs=wpb_sb[:], start=False, stop=True)
            ot = work.tile([P, D], F32, tag="ot")
            nc.vector.tensor_copy(ot[:], op[:])
            r0 = sc * SC + st * P
            nc.sync.dma_start(out_d[r0:r0 + P, :], ot[:])

        prev_av = None
        prev_avn = [None, None]
        for sc in range(N_SC):
            cur_av = None
            for g in range(NG):
                scores_group(sc, g)
                if prev_av is not None:
                    if g == 0:
                        prev_avn[0] = norm_head(prev_av[0], 0)
                    elif g == 1:
                        prev_avn[1] = norm_head(prev_av[1], 1)
                    elif 2 <= g < 6:
                        proj_st(sc - 1, prev_avn[0], prev_avn[1], g - 2)
                if g == 2:
                    avA = av_ps.tile([HD + 1, SC], F32, tag="avA", name="avA")
                    avB = av_ps.tile([HD + 1, SC], F32, tag="avB", name="avB")
                    cur_av = (avA, avB)
                if g >= 2:
                    av_group(cur_av[0], cur_av[1], g - 2)
            av_group(cur_av[0], cur_av[1], NG - 2)
            av_group(cur_av[0], cur_av[1], NG - 1)
            prev_av = cur_av
        avnA = norm_head(prev_av[0], 0)
        avnB = norm_head(prev_av[1], 1)
        for st in range(4):
            proj_st(N_SC - 1, avnA, avnB, st)


def build_nc():
    nc = bacc.Bacc("TRN2", target_bir_lowering=False, debug=False, num_devices=8)
    xT = nc.dram_tensor("xT", [D, S], BF16, kind="ExternalInput").ap()
    wq = nc.dram_tensor("wq", [D, P], BF16, kind="ExternalInput").ap()
    wk = nc.dram_tensor("wk", [D, P], BF16, kind="ExternalInput").ap()
    wv = nc.dram_tensor("wv", [D, VW], BF16, kind="ExternalInput").ap()
    bq = nc.dram_tensor("bq", [P, 1], F32, kind="ExternalInput").ap()
    bk = nc.dram_tensor("bk", [P, 1], F32, kind="ExternalInput").ap()
    bv = nc.dram_tensor("bv", [P, VW], F32, kind="ExternalInput").ap()
    wpa = nc.dram_tensor("wpa", [HD, D], BF16, kind="ExternalInput").ap()
    wpb = nc.dram_tensor("wpb", [HD, D], BF16, kind="ExternalInput").ap()
    out = nc.dram_tensor("out", [S, D], F32, kind="ExternalOutput").ap()
    with tile.TileContext(nc) as tc:
        _emit(tc, out, xT, wq, wk, wv, bq, bk, bv, wpa, wpb)
    nc.compile()
    return nc


def shard_inputs(x, W_qkv, b_qkv, W_proj):
    bf = ml_dtypes.bfloat16
    xTs = [np.ascontiguousarray(x[b].T).astype(bf) for b in range(B)]
    in_maps = []
    for c in range(8):
        b, hp = divmod(c, 4)
        h0 = 2 * hp
        qc = slice(h0 * HD, h0 * HD + P)
        kc = slice(D + h0 * HD, D + h0 * HD + P)
        v0 = 2 * D + h0 * HD
        wv = np.zeros((D, VW), np.float32)
        wv[:, 0:HD] = W_qkv[:, v0:v0 + HD]
        wv[:, HD + 1:2 * HD + 1] = W_qkv[:, v0 + HD:v0 + 2 * HD]
        bv = np.zeros((VW,), np.float32)
        bv[0:HD] = b_qkv[v0:v0 + HD]
        bv[HD] = 1.0
        bv[HD + 1:2 * HD + 1] = b_qkv[v0 + HD:v0 + 2 * HD]
        bv[VW - 1] = 1.0
        in_maps.append({
            "xT": xTs[b],
            "wq": np.ascontiguousarray(W_qkv[:, qc]).astype(bf),
            "wk": np.ascontiguousarray(W_qkv[:, kc]).astype(bf),
            "wv": wv.astype(bf),
            "bq": np.ascontiguousarray(b_qkv[qc]).reshape(P, 1).astype(np.float32),
            "bk": np.ascontiguousarray(b_qkv[kc]).reshape(P, 1).astype(np.float32),
            "bv": np.tile(bv[None, :], (P, 1)).astype(np.float32),
            "wpa": np.ascontiguousarray(W_proj[hp * P:hp * P + HD, :]).astype(bf),
            "wpb": np.ascontiguousarray(W_proj[hp * P + HD:(hp + 1) * P, :]).astype(bf),
        })
    return in_maps


def kernel(x, W_qkv, b_qkv, W_proj, b_proj):
    x = np.asarray(x, np.float32)
    W_qkv = np.asarray(W_qkv, np.float32)
    b_qkv = np.asarray(b_qkv, np.float32)
    W_proj = np.asarray(W_proj, np.float32)
    b_proj = np.asarray(b_proj, np.float32)

    global _NC
    if _NC is None:
        _NC = build_nc()
    in_maps = shard_inputs(x, W_qkv, b_qkv, W_proj)
    res = run_bass_kernel_spmd(_NC, in_maps, core_ids=list(range(8)))
    outs = [r["out"].astype(np.float32) for r in res.results]
    full = np.stack([outs[4 * b] + outs[4 * b + 1] + outs[4 * b + 2] + outs[4 * b + 3] + b_proj
                     for b in range(B)])
    return full.astype(np.float32)


# revision 7
# speedup vs baseline: 434.4698x; 434.4698x over previous
"""Multi-head attention (QKV proj + softmax attention + out proj) on 8 TRN2 NeuronCores.

Sharding: batch (2) x head-pairs (4) -> 8 cores. Each core computes q,k,v for its
2 heads of its batch, full attention over the 4096-token sequence for those heads,
and a partial output projection (row-sharded W_proj). The host sums the 4 partial
projections per batch and adds b_proj.

Per-core layout choices:
  - scores are computed transposed ([t, s] = keys on partitions), so the exp'd
    probabilities can feed the AV matmul directly as lhsT with no transposes.
  - the softmax denominator comes for free from a ones-column appended to V
    (row 64 of the [65, s] AV accumulator).
  - the two heads' score matmuls run concurrently on PE row-groups 0-1/2-3
    (K=64 each, lhsT/rhs at base partitions 0 and 64).
  - exp runs on ScalarE straight out of PSUM in [128, 1024] chunks, double
    buffered so ScalarE (the bottleneck engine) never waits on PE.
  - the qkv projections are interleaved into the first s-chunk's score loop so
    ScalarE starts almost immediately.
"""
from contextlib import ExitStack

import ml_dtypes
import numpy as np

import concourse.bass as bass
import concourse.tile as tile
from concourse import bacc, mybir
from concourse.bass_utils import run_bass_kernel_spmd

B, S, D = 2, 4096, 512
H, HD = 8, 64
SCALE = HD**-0.5
P = 128
SC = 512            # s-chunk width (query positions per inner block)
N_SC = S // SC      # 8
N_T = S // P        # 32 key chunks
KT = D // P         # 4 contraction tiles for the projections
VW = 2 * (HD + 1)   # v_ext width: [vA | onesA | vB | onesB]
BF16 = mybir.dt.bfloat16
F32 = mybir.dt.float32
EXP = mybir.ActivationFunctionType.Exp

_NC = None


def _emit(tc, out_d, xT_d, wq_d, wk_d, wv_d, bq_d, bk_d, bv_d, wpa_d, wpb_d):
    nc = tc.nc
    with ExitStack() as ctx:
        consts = ctx.enter_context(tc.tile_pool(name="consts", bufs=1))
        big = ctx.enter_context(tc.tile_pool(name="big", bufs=1))
        xt_pool = ctx.enter_context(tc.tile_pool(name="xt", bufs=3))
        work = ctx.enter_context(tc.tile_pool(name="work", bufs=2))

        wq_sb = consts.tile([P, KT, P], BF16, tag="wq")
        nc.sync.dma_start(wq_sb[:], wq_d.rearrange("(kt p) m -> p kt m", p=P))
        wk_sb = consts.tile([P, KT, P], BF16, tag="wk")
        nc.sync.dma_start(wk_sb[:], wk_d.rearrange("(kt p) m -> p kt m", p=P))
        wv_sb = consts.tile([P, KT, VW], BF16, tag="wv")
        nc.sync.dma_start(wv_sb[:], wv_d.rearrange("(kt p) m -> p kt m", p=P))
        wpa_sb = consts.tile([HD, D], BF16, tag="wpa")
        nc.sync.dma_start(wpa_sb[:], wpa_d)
        wpb_sb = consts.tile([HD, D], BF16, tag="wpb")
        nc.sync.dma_start(wpb_sb[:], wpb_d)
        bq_sb = consts.tile([P, 1], F32, tag="bq")
        nc.sync.dma_start(bq_sb[:], bq_d)
        bk_sb = consts.tile([P, 1], F32, tag="bk")
        nc.sync.dma_start(bk_sb[:], bk_d)
        bv_sb = consts.tile([P, VW], F32, tag="bv")
        nc.sync.dma_start(bv_sb[:], bv_d)
        # row HD (=64) of this is the lhsT of the K=1 broadcast matmul
        ones_sb = consts.tile([P, HD], F32, tag="ones")
        nc.vector.memset(ones_sb[:], 1.0)

        qT_sb = big.tile([P, S], BF16, tag="qT")   # rows 0-63 head A, 64-127 head B
        kT_sb = big.tile([P, S], BF16, tag="kT")
        v_sb = big.tile([P, N_T, VW], BF16, tag="v")
        pAB = big.tile([P, N_T, 2, SC], BF16, tag="p")  # exp'd scoresT per head

        xT_r = xT_d.rearrange("(kt p) s -> p kt s", p=P)

        # PSUM pools: scores 2x2 banks, AV accumulators 2 banks, misc 2 banks
        # (misc doubles as the qkv-projection psum during the fused phase 1).
        sc_ps = ctx.enter_context(tc.tile_pool(name="scps", bufs=2, space="PSUM"))
        av_ps = ctx.enter_context(tc.tile_pool(name="avps", bufs=1, space="PSUM"))
        mi_ps = ctx.enter_context(tc.tile_pool(name="mips", bufs=2, space="PSUM"))

        def load_x(c, tag="xt"):
            csl = slice(c * SC, (c + 1) * SC)
            xt = xt_pool.tile([P, KT, SC], BF16, tag=tag, name="xt")
            nc.sync.dma_start(xt[:], xT_r[:, :, csl])
            return xt

        def k_chunk(c, xt):
            csl = slice(c * SC, (c + 1) * SC)
            kp = mi_ps.tile([P, SC], F32, tag="m", name="kp")
            for kt in range(KT):
                nc.tensor.matmul(kp[:], lhsT=wk_sb[:, kt, :], rhs=xt[:, kt, :],
                                 start=kt == 0, stop=kt == KT - 1)
            nc.vector.tensor_scalar_add(out=kT_sb[:, csl], in0=kp[:], scalar1=bk_sb[:])

        def q_chunk(c, xt):
            csl = slice(c * SC, (c + 1) * SC)
            qp = mi_ps.tile([P, SC], F32, tag="m", name="qp")
            for kt in range(KT):
                nc.tensor.matmul(qp[:], lhsT=wq_sb[:, kt, :], rhs=xt[:, kt, :],
                                 start=kt == 0, stop=kt == KT - 1)
            nc.vector.tensor_scalar_add(out=qT_sb[:, csl], in0=qp[:], scalar1=bq_sb[:])

        def v_chunk(c, xt, pair):
            for st in (2 * pair, 2 * pair + 1):
                vp = mi_ps.tile([P, VW], F32, tag="m", name="vp")
                for kt in range(KT):
                    nc.tensor.matmul(vp[:], lhsT=xt[:, kt, st * P:(st + 1) * P],
                                     rhs=wv_sb[:, kt, :],
                                     start=kt == 0, stop=kt == KT - 1)
                nc.vector.tensor_add(out=v_sb[:, c * 4 + st, :], in0=vp[:], in1=bv_sb[:])

        def scores_t(sc, t):
            ssl = slice(sc * SC, (sc + 1) * SC)
            tsl = slice(t * P, (t + 1) * P)
            sAB = sc_ps.tile([P, 2, SC], F32, tag="s", name="sAB")
            nc.tensor.matmul(sAB[:, 0, :], lhsT=kT_sb[0:HD, tsl],
                             rhs=qT_sb[0:HD, ssl], start=True, stop=True)
            nc.tensor.matmul(sAB[:, 1, :], lhsT=kT_sb[HD:P, tsl],
                             rhs=qT_sb[HD:P, ssl], start=True, stop=True)
            nc.scalar.activation(out=pAB[:, t, :, :], in_=sAB[:], func=EXP, scale=SCALE)

        def av_t(avA, avB, t):
            nc.tensor.matmul(avA[:], lhsT=v_sb[:, t, 0:HD + 1], rhs=pAB[:, t, 0, :],
                             start=t == 0, stop=t == N_T - 1)
            nc.tensor.matmul(avB[:], lhsT=v_sb[:, t, HD + 1:VW], rhs=pAB[:, t, 1, :],
                             start=t == 0, stop=t == N_T - 1)

        def av_evac(avX, h):
            # evacuate the [65, SC] psum accumulator to sbuf so the bank frees early
            avs = work.tile([HD + 1, SC], F32, tag=f"avs{h}")
            nc.vector.tensor_copy(avs[:], avX[:])
            return avs

        def norm_head(avs, h):
            # avs: [65, SC] sbuf f32; row 64 = sum_t exp. Result: [64, SC] bf16.
            rec = work.tile([HD + 1, SC], F32, tag="rec")
            nc.vector.reciprocal(rec[HD:HD + 1, :], avs[HD:HD + 1, :])
            bc_ps = mi_ps.tile([HD, SC], F32, tag="m", name="bc_ps")
            nc.tensor.matmul(bc_ps[:], lhsT=ones_sb[HD:HD + 1, :], rhs=rec[HD:HD + 1, :],
                             start=True, stop=True)
            bc_sb = work.tile([HD, SC], F32, tag="bc")
            nc.vector.tensor_copy(bc_sb[:], bc_ps[:])
            avn = work.tile([HD, SC], BF16, tag=f"avn{h}")
            nc.vector.tensor_mul(avn[:], avs[0:HD, :], bc_sb[:])
            return avn

        def proj_st(sc, avnA, avnB, st):
            op = mi_ps.tile([P, D], F32, tag="m", name="op")
            asl = slice(st * P, (st + 1) * P)
            nc.tensor.matmul(op[:], lhsT=avnA[:, asl], rhs=wpa_sb[:], start=True, stop=False)
            nc.tensor.matmul(op[:], lhsT=avnB[:, asl], rhs=wpb_sb[:], start=False, stop=True)
            ot = work.tile([P, D], F32, tag="ot")
            nc.vector.tensor_copy(ot[:], op[:])
            r0 = sc * SC + st * P
            nc.sync.dma_start(out_d[r0:r0 + P, :], ot[:])

        LAG = 4  # av group t' = t - LAG runs at score step t

        # PE warm-up burst: junk matmuls that run while the first x DMA is in
        # flight, so the HAM clock-gate is at full rate when real work starts.
        for w in range(10):
            warm = mi_ps.tile([HD, HD], F32, tag="m", name="warm")
            nc.tensor.matmul(warm[:], lhsT=ones_sb[:, 0:HD], rhs=ones_sb[:, 0:HD],
                             start=True, stop=True)

        # prologue: k, q, v for chunk 0 only; the rest streams into the sweep
        xt0 = load_x(0)
        k_chunk(0, xt0)
        q_chunk(0, xt0)
        v_chunk(0, xt0, 0)
        v_chunk(0, xt0, 1)

        prev_av = None      # previous s-chunk's (avA, avB) psum accumulators
        prev_avs = [None, None]
        prev_avn = [None, None]
        xt_cur = None
        xtq = None
        for sc in range(N_SC):
            cur_av = None
            for t in range(N_T):
                # remaining k/v chunks stream in during sc=0's sweep
                if sc == 0:
                    c = t // 4 + 1
                    if c <= N_SC - 1:
                        if t % 4 == 0:
                            xt_cur = load_x(c)
                            k_chunk(c, xt_cur)
                        elif t % 4 == 1:
                            v_chunk(c, xt_cur, 0)
                        elif t % 4 == 2:
                            v_chunk(c, xt_cur, 1)
                # next s-chunk's q projection (needs a fresh x load)
                if sc < N_SC - 1:
                    if t == 16:
                        xtq = load_x(sc + 1, tag="xtq")
                    elif t == 20:
                        q_chunk(sc + 1, xtq)
                scores_t(sc, t)
                if prev_av is not None:
                    if t < 2:
                        av_t(prev_av[0], prev_av[1], N_T - LAG + t)
                    elif t == 2:
                        av_t(prev_av[0], prev_av[1], N_T - 2)
                    elif t == 3:
                        av_t(prev_av[0], prev_av[1], N_T - 1)
                        prev_avs[0] = av_evac(prev_av[0], 0)
                        prev_avs[1] = av_evac(prev_av[1], 1)
                    elif t == 4:
                        prev_avn[0] = norm_head(prev_avs[0], 0)
                    elif t == 5:
                        prev_avn[1] = norm_head(prev_avs[1], 1)
                    elif 6 <= t < 10:
                        proj_st(sc - 1, prev_avn[0], prev_avn[1], t - 6)
                if t == LAG:
                    avA = av_ps.tile([HD + 1, SC], F32, tag="avA", name="avA")
                    avB = av_ps.tile([HD + 1, SC], F32, tag="avB", name="avB")
                    cur_av = (avA, avB)
                if t >= LAG:
                    av_t(cur_av[0], cur_av[1], t - LAG)
            prev_av = cur_av
        for t in range(LAG):
            av_t(prev_av[0], prev_av[1], N_T - LAG + t)
        avsA = av_evac(prev_av[0], 0)
        avsB = av_evac(prev_av[1], 1)
        avnA = norm_head(avsA, 0)
        avnB = norm_head(avsB, 1)
        for st in range(4):
            proj_st(N_SC - 1, avnA, avnB, st)


def build_nc():
    nc = bacc.Bacc("TRN2", target_bir_lowering=False, debug=False, num_devices=8)
    xT = nc.dram_tensor("xT", [D, S], BF16, kind="ExternalInput").ap()
    wq = nc.dram_tensor("wq", [D, P], BF16, kind="ExternalInput").ap()
    wk = nc.dram_tensor("wk", [D, P], BF16, kind="ExternalInput").ap()
    wv = nc.dram_tensor("wv", [D, VW], BF16, kind="ExternalInput").ap()
    bq = nc.dram_tensor("bq", [P, 1], F32, kind="ExternalInput").ap()
    bk = nc.dram_tensor("bk", [P, 1], F32, kind="ExternalInput").ap()
    bv = nc.dram_tensor("bv", [P, VW], F32, kind="ExternalInput").ap()
    wpa = nc.dram_tensor("wpa", [HD, D], BF16, kind="ExternalInput").ap()
    wpb = nc.dram_tensor("wpb", [HD, D], BF16, kind="ExternalInput").ap()
    out = nc.dram_tensor("out", [S, D], F32, kind="ExternalOutput").ap()
    with tile.TileContext(nc) as tc:
        _emit(tc, out, xT, wq, wk, wv, bq, bk, bv, wpa, wpb)
    nc.compile()
    return nc


def shard_inputs(x, W_qkv, b_qkv, W_proj):
    bf = ml_dtypes.bfloat16
    xTs = [np.ascontiguousarray(x[b].T).astype(bf) for b in range(B)]
    in_maps = []
    for c in range(8):
        b, hp = divmod(c, 4)
        h0 = 2 * hp
        qc = slice(h0 * HD, h0 * HD + P)
        kc = slice(D + h0 * HD, D + h0 * HD + P)
        v0 = 2 * D + h0 * HD
        wv = np.zeros((D, VW), np.float32)
        wv[:, 0:HD] = W_qkv[:, v0:v0 + HD]
        wv[:, HD + 1:2 * HD + 1] = W_qkv[:, v0 + HD:v0 + 2 * HD]
        bv = np.zeros((VW,), np.float32)
        bv[0:HD] = b_qkv[v0:v0 + HD]
        bv[HD] = 1.0
        bv[HD + 1:2 * HD + 1] = b_qkv[v0 + HD:v0 + 2 * HD]
        bv[VW - 1] = 1.0
        in_maps.append({
            "xT": xTs[b],
            "wq": np.ascontiguousarray(W_qkv[:, qc]).astype(bf),
            "wk": np.ascontiguousarray(W_qkv[:, kc]).astype(bf),
            "wv": wv.astype(bf),
            "bq": np.ascontiguousarray(b_qkv[qc]).reshape(P, 1).astype(np.float32),
            "bk": np.ascontiguousarray(b_qkv[kc]).reshape(P, 1).astype(np.float32),
            "bv": np.tile(bv[None, :], (P, 1)).astype(np.float32),
            "wpa": np.ascontiguousarray(W_proj[hp * P:hp * P + HD, :]).astype(bf),
            "wpb": np.ascontiguousarray(W_proj[hp * P + HD:(hp + 1) * P, :]).astype(bf),
        })
    return in_maps


def kernel(x, W_qkv, b_qkv, W_proj, b_proj):
    x = np.asarray(x, np.float32)
    W_qkv = np.asarray(W_qkv, np.float32)
    b_qkv = np.asarray(b_qkv, np.float32)
    W_proj = np.asarray(W_proj, np.float32)
    b_proj = np.asarray(b_proj, np.float32)

    global _NC
    if _NC is None:
        _NC = build_nc()
    in_maps = shard_inputs(x, W_qkv, b_qkv, W_proj)
    res = run_bass_kernel_spmd(_NC, in_maps, core_ids=list(range(8)))
    outs = [r["out"].astype(np.float32) for r in res.results]
    full = np.stack([outs[4 * b] + outs[4 * b + 1] + outs[4 * b + 2] + outs[4 * b + 3] + b_proj
                     for b in range(B)])
    return full.astype(np.float32)


# revision 20
# speedup vs baseline: 4516.2633x; 10.3949x over previous
"""Multi-head attention (QKV proj + softmax attention + out proj) on 8 TRN2 NeuronCores.

Sharding: batch (2) x head-pairs (4) -> 8 cores. Each core computes q,k,v for its
2 heads of its batch, full attention over the 4096-token sequence for those heads,
and a partial output projection (row-sharded W_proj). The host sums the 4 partial
projections per batch and adds b_proj.

Per-core layout choices:
  - scores are computed transposed ([t, s] = keys on partitions), so the exp'd
    probabilities can feed the AV matmul directly as lhsT with no transposes.
  - the softmax denominator comes for free from a ones-column appended to V
    (row 64 of the [65, s] AV accumulator).
  - the two heads' score matmuls run concurrently on PE row-groups 0-1/2-3
    (K=64 each, lhsT/rhs at base partitions 0 and 64).
  - exp runs on ScalarE straight out of PSUM in [128, 1024] chunks, double
    buffered so ScalarE (the bottleneck engine) never waits on PE.
  - the qkv projections are interleaved into the first s-chunk's score loop so
    ScalarE starts almost immediately.
"""
from contextlib import ExitStack

import ml_dtypes
import numpy as np

import concourse.bass as bass
import concourse.tile as tile
from concourse import bacc, mybir
from concourse.bass_utils import run_bass_kernel_spmd

B, S, D = 2, 4096, 512
H, HD = 8, 64
SCALE = HD**-0.5
P = 128
SC = 512            # s-chunk width (query positions per inner block)
N_SC = S // SC      # 8
N_T = S // P        # 32 key chunks
KT = D // P         # 4 contraction tiles for the projections
VW = 144            # v_ext row width, 16B-aligned stride: [vA|1A @0..64, pad, vB|1B @72..136]
VB0 = 72            # head-B column offset inside v_ext
BF16 = mybir.dt.bfloat16
F32 = mybir.dt.float32
EXP = mybir.ActivationFunctionType.Exp

_NC = None


def _emit(tc, out_d, xT_d, wq_d, wk_d, wv_d, bq_d, bk_d, bv_d, wpa_d, wpb_d, reps=1, hw_loop=0, exp_half=False, score_double=False, av_half=False):
    nc = tc.nc
    with ExitStack() as ctx:
        consts = ctx.enter_context(tc.tile_pool(name="consts", bufs=1))
        big = ctx.enter_context(tc.tile_pool(name="big", bufs=1))
        xt_pool = ctx.enter_context(tc.tile_pool(name="xt", bufs=3))
        work = ctx.enter_context(tc.tile_pool(name="work", bufs=2))

        wq_sb = consts.tile([P, KT, P], BF16, tag="wq")
        nc.sync.dma_start(wq_sb[:], wq_d.rearrange("(kt p) m -> p kt m", p=P))
        wk_sb = consts.tile([P, KT, P], BF16, tag="wk")
        nc.sync.dma_start(wk_sb[:], wk_d.rearrange("(kt p) m -> p kt m", p=P))
        wv_sb = consts.tile([P, KT, VW], BF16, tag="wv")
        nc.sync.dma_start(wv_sb[:], wv_d.rearrange("(kt p) m -> p kt m", p=P))
        wpa_sb = consts.tile([HD, D], BF16, tag="wpa")
        nc.sync.dma_start(wpa_sb[:], wpa_d)
        wpb_sb = consts.tile([HD, D], BF16, tag="wpb")
        nc.sync.dma_start(wpb_sb[:], wpb_d)
        bq_sb = consts.tile([P, 1], F32, tag="bq")
        nc.sync.dma_start(bq_sb[:], bq_d)
        bk_sb = consts.tile([P, 1], F32, tag="bk")
        nc.sync.dma_start(bk_sb[:], bk_d)
        bv_sb = consts.tile([P, VW], F32, tag="bv")
        nc.sync.dma_start(bv_sb[:], bv_d)
        # row HD (=64) of this is the lhsT of the K=1 broadcast matmul
        ones_sb = consts.tile([P, HD], F32, tag="ones")
        nc.vector.memset(ones_sb[:], 1.0)

        qT_sb = big.tile([P, S], BF16, tag="qT")   # rows 0-63 head A, 64-127 head B
        kT_sb = big.tile([P, S], BF16, tag="kT")
        v_sb = big.tile([P, N_T, VW], BF16, tag="v")
        pAB = big.tile([P, N_T, 2, SC], BF16, tag="p")  # exp'd scoresT per head
        if exp_half:
            nc.vector.memset(pAB[:], 1.0)

        xT_r = xT_d.rearrange("(kt p) s -> p kt s", p=P)

        # PSUM pools: scores 2x2 banks, AV accumulators 2 banks, misc 2 banks
        # (misc doubles as the qkv-projection psum during the fused phase 1).
        sc_ps = ctx.enter_context(tc.tile_pool(name="scps", bufs=2, space="PSUM"))
        av_ps = ctx.enter_context(tc.tile_pool(name="avps", bufs=1, space="PSUM"))
        mi_ps = ctx.enter_context(tc.tile_pool(name="mips", bufs=2, space="PSUM"))

        def load_x(c, tag="xt"):
            csl = slice(c * SC, (c + 1) * SC)
            xt = xt_pool.tile([P, KT, SC], BF16, tag=tag, name="xt")
            nc.sync.dma_start(xt[:], xT_r[:, :, csl])
            return xt

        def k_chunk(c, xt):
            csl = slice(c * SC, (c + 1) * SC)
            kp = mi_ps.tile([P, SC], F32, tag="m", name="kp")
            for kt in range(KT):
                nc.tensor.matmul(kp[:], lhsT=wk_sb[:, kt, :], rhs=xt[:, kt, :],
                                 start=kt == 0, stop=kt == KT - 1)
            nc.vector.tensor_scalar_add(out=kT_sb[:, csl], in0=kp[:], scalar1=bk_sb[:])

        def q_chunk(c, xt):
            csl = slice(c * SC, (c + 1) * SC)
            qp = mi_ps.tile([P, SC], F32, tag="m", name="qp")
            for kt in range(KT):
                nc.tensor.matmul(qp[:], lhsT=wq_sb[:, kt, :], rhs=xt[:, kt, :],
                                 start=kt == 0, stop=kt == KT - 1)
            nc.vector.tensor_scalar_add(out=qT_sb[:, csl], in0=qp[:], scalar1=bq_sb[:])

        def v_chunk(c, xt, pair):
            for st in (2 * pair, 2 * pair + 1):
                vp = mi_ps.tile([P, VW], F32, tag="m", name="vp")
                for kt in range(KT):
                    nc.tensor.matmul(vp[:], lhsT=xt[:, kt, st * P:(st + 1) * P],
                                     rhs=wv_sb[:, kt, :],
                                     start=kt == 0, stop=kt == KT - 1)
                nc.vector.tensor_add(out=v_sb[:, c * 4 + st, :], in0=vp[:], in1=bv_sb[:])

        def scores_t(sc, t):
            ssl = slice(sc * SC, (sc + 1) * SC)
            tsl = slice(t * P, (t + 1) * P)
            sAB = sc_ps.tile([P, 2, SC], F32, tag="s", name="sAB")
            n_mm = 2 if score_double else 1
            for _i in range(n_mm):
                nc.tensor.matmul(sAB[:, 0, :], lhsT=kT_sb[0:HD, tsl],
                                 rhs=qT_sb[0:HD, ssl], start=True, stop=True)
                nc.tensor.matmul(sAB[:, 1, :], lhsT=kT_sb[HD:P, tsl],
                                 rhs=qT_sb[HD:P, ssl], start=True, stop=True)
            if exp_half:
                nc.scalar.activation(out=pAB[:, t, :, 0:SC // 2],
                                     in_=sAB[:, :, 0:SC // 2], func=EXP, scale=SCALE)
            else:
                nc.scalar.activation(out=pAB[:, t, :, :], in_=sAB[:], func=EXP, scale=SCALE)

        def av_t(avA, avB, t):
            aw = SC // 2 if av_half else SC
            nc.tensor.matmul(avA[:, 0:aw], lhsT=v_sb[:, t, 0:HD + 1],
                             rhs=pAB[:, t, 0, 0:aw],
                             start=t == 0, stop=t == N_T - 1)
            nc.tensor.matmul(avB[:, 0:aw], lhsT=v_sb[:, t, VB0:VB0 + HD + 1],
                             rhs=pAB[:, t, 1, 0:aw],
                             start=t == 0, stop=t == N_T - 1)

        def av_evac(avX, h):
            # evacuate the [65, SC] psum accumulator to sbuf so the bank frees early
            avs = work.tile([HD + 1, SC], F32, tag=f"avs{h}")
            nc.vector.tensor_copy(avs[:], avX[:])
            return avs

        def norm_head(avs, h):
            # avs: [65, SC] sbuf f32; row 64 = sum_t exp. Result: [64, SC] bf16.
            rec = work.tile([HD + 1, SC], F32, tag="rec")
            nc.vector.reciprocal(rec[HD:HD + 1, :], avs[HD:HD + 1, :])
            bc_ps = mi_ps.tile([HD, SC], F32, tag="m", name="bc_ps")
            nc.tensor.matmul(bc_ps[:], lhsT=ones_sb[HD:HD + 1, :], rhs=rec[HD:HD + 1, :],
                             start=True, stop=True)
            bc_sb = work.tile([HD, SC], F32, tag="bc")
            nc.vector.tensor_copy(bc_sb[:], bc_ps[:])
            avn = work.tile([HD, SC], BF16, tag=f"avn{h}")
            nc.vector.tensor_mul(avn[:], avs[0:HD, :], bc_sb[:])
            return avn

        def proj_st(sc, avnA, avnB, st):
            op = mi_ps.tile([P, D], F32, tag="m", name="op")
            asl = slice(st * P, (st + 1) * P)
            nc.tensor.matmul(op[:], lhsT=avnA[:, asl], rhs=wpa_sb[:], start=True, stop=False)
            nc.tensor.matmul(op[:], lhsT=avnB[:, asl], rhs=wpb_sb[:], start=False, stop=True)
            ot = work.tile([P, D], F32, tag="ot")
            nc.vector.tensor_copy(ot[:], op[:])
            r0 = sc * SC + st * P
            nc.sync.dma_start(out_d[r0:r0 + P, :], ot[:])

        LAG = 4  # av group t' = t - LAG runs at score step t

        # PE warm-up burst: junk matmuls that run while the first x DMA is in
        # flight, so the HAM clock-gate is at full rate when real work starts.
        for w in range(10):
            warm = mi_ps.tile([HD, HD], F32, tag="m", name="warm")
            nc.tensor.matmul(warm[:], lhsT=ones_sb[:, 0:HD], rhs=ones_sb[:, 0:HD],
                             start=True, stop=True)

        if hw_loop:
            with tc.For_i(0, hw_loop, 1):
                _emit_body(tc, out_d, xT_r, load_x, k_chunk, q_chunk, v_chunk,
                           scores_t, av_t, av_evac, norm_head, proj_st, av_ps, LAG)
        else:
            for _rep in range(reps):
                _emit_body(tc, out_d, xT_r, load_x, k_chunk, q_chunk, v_chunk,
                           scores_t, av_t, av_evac, norm_head, proj_st, av_ps, LAG)


def _emit_body(tc, out_d, xT_r, load_x, k_chunk, q_chunk, v_chunk, scores_t,
               av_t, av_evac, norm_head, proj_st, av_ps, LAG):
        nc = tc.nc
        # prologue: k, q, v for chunk 0; chunk 1's x prefetched right away
        xt0 = load_x(0)
        xt_nxt = load_x(1)
        k_chunk(0, xt0)
        q_chunk(0, xt0)
        v_chunk(0, xt0, 0)
        v_chunk(0, xt0, 1)

        prev_av = None      # previous s-chunk's (avA, avB) psum accumulators
        prev_avs = [None, None]
        prev_avn = [None, None]
        xt_cur = None
        xtq = None
        for sc in range(N_SC):
            cur_av = None
            for t in range(N_T):
                # remaining k/v chunks stream in during sc=0's sweep
                if sc == 0:
                    c = t // 4 + 1
                    if c <= N_SC - 1:
                        if t % 4 == 0:
                            xt_cur = xt_nxt
                            k_chunk(c, xt_cur)
                        elif t % 4 == 1:
                            v_chunk(c, xt_cur, 0)
                        elif t % 4 == 2:
                            v_chunk(c, xt_cur, 1)
                            if c + 1 <= N_SC - 1:
                                xt_nxt = load_x(c + 1)
                # next s-chunk's q projection (needs a fresh x load)
                if sc < N_SC - 1:
                    if t == 16:
                        xtq = load_x(sc + 1, tag="xtq")
                    elif t == 20:
                        q_chunk(sc + 1, xtq)
                scores_t(sc, t)
                if prev_av is not None:
                    if t < LAG:
                        av_t(prev_av[0], prev_av[1], N_T - LAG + t)
                        if t == LAG - 1:
                            prev_avs[0] = av_evac(prev_av[0], 0)
                            prev_avs[1] = av_evac(prev_av[1], 1)
                    elif t == LAG:
                        prev_avn[0] = norm_head(prev_avs[0], 0)
                    elif t == LAG + 1:
                        prev_avn[1] = norm_head(prev_avs[1], 1)
                    elif LAG + 2 <= t < LAG + 6:
                        proj_st(sc - 1, prev_avn[0], prev_avn[1], t - LAG - 2)
                if t == LAG:
                    avA = av_ps.tile([HD + 1, SC], F32, tag="avA", name="avA")
                    avB = av_ps.tile([HD + 1, SC], F32, tag="avB", name="avB")
                    cur_av = (avA, avB)
                if t >= LAG:
                    av_t(cur_av[0], cur_av[1], t - LAG)
            prev_av = cur_av
        for t in range(LAG):
            av_t(prev_av[0], prev_av[1], N_T - LAG + t)
        avsA = av_evac(prev_av[0], 0)
        avsB = av_evac(prev_av[1], 1)
        avnA = norm_head(avsA, 0)
        avnB = norm_head(avsB, 1)
        for st in range(4):
            proj_st(N_SC - 1, avnA, avnB, st)


def build_nc(reps=1, hw_loop=0, exp_half=False, score_double=False, av_half=False):
    nc = bacc.Bacc("TRN2", target_bir_lowering=False, debug=False, num_devices=8)
    xT = nc.dram_tensor("xT", [D, S], BF16, kind="ExternalInput").ap()
    wq = nc.dram_tensor("wq", [D, P], BF16, kind="ExternalInput").ap()
    wk = nc.dram_tensor("wk", [D, P], BF16, kind="ExternalInput").ap()
    wv = nc.dram_tensor("wv", [D, VW], BF16, kind="ExternalInput").ap()
    bq = nc.dram_tensor("bq", [P, 1], F32, kind="ExternalInput").ap()
    bk = nc.dram_tensor("bk", [P, 1], F32, kind="ExternalInput").ap()
    bv = nc.dram_tensor("bv", [P, VW], F32, kind="ExternalInput").ap()
    wpa = nc.dram_tensor("wpa", [HD, D], BF16, kind="ExternalInput").ap()
    wpb = nc.dram_tensor("wpb", [HD, D], BF16, kind="ExternalInput").ap()
    out = nc.dram_tensor("out", [S, D], F32, kind="ExternalOutput").ap()
    with tile.TileContext(nc) as tc:
        _emit(tc, out, xT, wq, wk, wv, bq, bk, bv, wpa, wpb, reps=reps, hw_loop=hw_loop, exp_half=exp_half, score_double=score_double, av_half=av_half)
    nc.compile()
    return nc


def shard_inputs(x, W_qkv, b_qkv, W_proj):
    bf = ml_dtypes.bfloat16
    xTs = [np.ascontiguousarray(x[b].T).astype(bf) for b in range(B)]
    in_maps = []
    for c in range(8):
        b, hp = divmod(c, 4)
        h0 = 2 * hp
        qc = slice(h0 * HD, h0 * HD + P)
        kc = slice(D + h0 * HD, D + h0 * HD + P)
        v0 = 2 * D + h0 * HD
        wv = np.zeros((D, VW), np.float32)
        wv[:, 0:HD] = W_qkv[:, v0:v0 + HD]
        wv[:, VB0:VB0 + HD] = W_qkv[:, v0 + HD:v0 + 2 * HD]
        bv = np.zeros((VW,), np.float32)
        bv[0:HD] = b_qkv[v0:v0 + HD]
        bv[HD] = 1.0
        bv[VB0:VB0 + HD] = b_qkv[v0 + HD:v0 + 2 * HD]
        bv[VB0 + HD] = 1.0
        in_maps.append({
            "xT": xTs[b],
            "wq": np.ascontiguousarray(W_qkv[:, qc]).astype(bf),
            "wk": np.ascontiguousarray(W_qkv[:, kc]).astype(bf),
            "wv": wv.astype(bf),
            "bq": np.ascontiguousarray(b_qkv[qc]).reshape(P, 1).astype(np.float32),
            "bk": np.ascontiguousarray(b_qkv[kc]).reshape(P, 1).astype(np.float32),
            "bv": np.tile(bv[None, :], (P, 1)).astype(np.float32),
            "wpa": np.ascontiguousarray(W_proj[hp * P:hp * P + HD, :]).astype(bf),
            "wpb": np.ascontiguousarray(W_proj[hp * P + HD:(hp + 1) * P, :]).astype(bf),
        })
    return in_maps


def kernel(x, W_qkv, b_qkv, W_proj, b_proj):
    x = np.asarray(x, np.float32)
    W_qkv = np.asarray(W_qkv, np.float32)
    b_qkv = np.asarray(b_qkv, np.float32)
    W_proj = np.asarray(W_proj, np.float32)
    b_proj = np.asarray(b_proj, np.float32)

    global _NC
    if _NC is None:
        _NC = build_nc()
    in_maps = shard_inputs(x, W_qkv, b_qkv, W_proj)
    res = run_bass_kernel_spmd(_NC, in_maps, core_ids=list(range(8)))
    outs = [r["out"].astype(np.float32) for r in res.results]
    full = np.stack([outs[4 * b] + outs[4 * b + 1] + outs[4 * b + 2] + outs[4 * b + 3] + b_proj
                     for b in range(B)])
    return full.astype(np.float32)
